# revision 31
# baseline (speedup 1.0000x reference)
"""Trainium2 Bass kernel for nn_CatConLayers (multi-head cross-attention over
time/category embeddings).

Sharding: 8 cores = 4 batches x 2 head-pairs. Each core computes, for its
batch b and heads {2g, 2g+1}:
  s_c^T = k_in^T-chunk-c @ [m_0|m_1]    (kT chunk stationary, heads batched;
                                         m_h = (Wq_h@Wk_h^T)^T q_in^T + Wk_h@bq_h
                                         is pure weight preprocessing, host-fused)
  p~    = exp(s/sqrt(KQ))               (scores are tiny: no max-subtraction;
                                         the bk term cancels in the softmax)
  vo    = sum_c x_c^T @ p~_c            (value matmul, PSUM accumulation)
  Z     = ones^T @ p~ partial rows -> fp16 row, transposed to columns by two
          tiny matmuls -> 1/Z columns
  out_h = (vo_h / Z_h) @ Wo_h           (normalization folded after Wo)
Host: builds k_in^T featurization (sinusoidal time embedding + category-
embedding rows; the ACT Sin table cannot be co-resident with the Exp table,
and on-device indirect-DMA gathers measured 1.1us each), computes ms (weights
x fixed reference queries only), shards inputs, sums the two head-pair
partials per batch, adds bo.

Schedule notes (from NTFF traces): the kernel is input-DMA-latency bound at
the start, so kT is split across both HWDGE queues (sync+scalar) and issued
first; ms rides behind one half; x rides SWDGE (naturally later, needed
later). Outputs are two independent fp16 tiles on the two HWDGE queues so
neither waits on the other. All matmul operands fp16, fp32 PSUM accumulate.

The KQ dimension is permuted (sin block | cos block | emb0 | emb1) so the
interleaved sin/cos layout of the reference never has to be materialized
on-chip; Wq/Wk rows are permuted identically on host before the ms fusion.
"""

import numpy as np

import concourse.bass as bass
import concourse.mybir as mybir
import concourse.tile as tile
from concourse import bacc
from concourse.bass_utils import run_bass_kernel_spmd

# Problem shapes (hardcoded per harness contract)
N, T, H, KQ, LD, NREF, DT = 4, 1024, 4, 128, 128, 128, 64
NCORES = 8
TCH = T // 128  # 8 key chunks of 128

F32 = mybir.dt.float32
FP16 = mybir.dt.float16
AF = mybir.ActivationFunctionType

VALUE_DTYPE = "f16"  # kept for test.py compatibility

# PE warmup matmuls (N=512 each, ~427ns cold): keep the PE busy through the
# input-DMA window so the HAM clock-gate is released before the real matmuls.
WARM_CNT = 5

_CACHE = {}


def _build_program(vd_name="f16"):
    SD = VD = FP16
    nc = bacc.Bacc("TRN2", target_bir_lowering=False, debug=False,
                   num_devices=NCORES)

    # inputs: kT split across the two HWDGE queues; ms tiny behind one half;
    # x||wo on SWDGE (needed only after the first exp).
    # mk = [ms | kt chunks 4-7] — the whole critical first score group as a
    # single DMA on one queue; kt0 (chunks 0-3) arrives second via SWDGE.
    mk_d = nc.dram_tensor("mk", [KQ, 2 * NREF + T // 2], SD,
                          kind="ExternalInput")
    kt0_d = nc.dram_tensor("kt0", [KQ, T // 2], SD, kind="ExternalInput")
    xw_d = nc.dram_tensor("xw", [128, T], VD, kind="ExternalInput")
    wo_d = nc.dram_tensor("wo", [LD, 2 * LD], VD, kind="ExternalInput")
    out_d = nc.dram_tensor("out", [NREF, 2 * LD], FP16, kind="ExternalOutput")

    inv_sqrt_kq = float(1.0 / np.sqrt(KQ))

    with tile.TileContext(nc) as tc:
        # Everything is single-use: one const SBUF pool (stable addresses, no
        # ring-overlap deps) + PSUM. PSUM banks (8 x 2KB): sc 2x2 (shared
        # with the warmup tile), zrow 1, vo|zc 1, fin0 1, fin1 1.
        with tc.tile_pool(name="const", bufs=1) as cp, \
             tc.tile_pool(name="ps", bufs=2, space="PSUM") as pp:

            # ---- input DMAs. All in-flight DMA queues share the SDMA
            # engines round-robin at packet granularity, so anything issued
            # early steals bandwidth from the critical path. Only ms (sync)
            # + kt1 (scalar) go immediately; kt0/x/wo descriptor generation
            # on SWDGE is held back behind a ~1.3us dummy memset so their
            # transfers start about when kt1 finishes.
            mk = cp.tile([KQ, 2 * NREF + T // 2], SD)
            nc.scalar.dma_start(out=mk[:], in_=mk_d[:])
            ms = mk[:, 0:2 * NREF]
            delay = cp.tile([128, 1536], F32)
            nc.gpsimd.memset(delay[:], 0.0)
            kTb = cp.tile([KQ, T // 2], SD)
            nc.gpsimd.dma_start(out=kTb[:], in_=kt0_d[:])
            xw = cp.tile([128, T], VD)
            nc.gpsimd.dma_start(out=xw[:], in_=xw_d[:])
            wo = cp.tile([LD, 2 * LD], VD)
            nc.gpsimd.dma_start(out=wo[:], in_=wo_d[:])

            def kchunk(c):  # kT columns for key chunk c
                if c >= 4:
                    return mk[:, 2 * NREF + (c - 4) * 128:
                              2 * NREF + (c - 3) * 128]
                return kTb[:, c * 128:(c + 1) * 128]

            # warmup streams a stride-0 broadcast of the tiny ones column —
            # no big memset, so the PE goes busy ~0.25us earlier and the HAM
            # clock-gate releases before the score matmuls. A final N=256
            # bridge seams into the mk arrival without an idle gap.
            ones_col = cp.tile([128, 1], VD)
            nc.vector.memset(ones_col[:], 1.0)
            one11 = cp.tile([1, 1], FP16)
            nc.vector.memset(one11[:], 1.0)
            wlhs = ones_col[:, 0:1].broadcast_to([128, 128])
            wrhs = ones_col[:, 0:1].broadcast_to([128, 512])
            wps = pp.tile([128, 1024], F32, tag="sc", bufs=2)
            for i in range(WARM_CNT):
                nc.tensor.matmul(out=wps[:, 0:512], lhsT=wlhs,
                                 rhs=wrhs, start=True, stop=True)
            nc.tensor.matmul(out=wps[:, 0:256], lhsT=wlhs,
                             rhs=ones_col[:, 0:1].broadcast_to([128, 256]),
                             start=True, stop=True)

            # ---- scores^T + exp. p~^T layout: chunk c, head h at
            # pT_all[:, c*256 + h*128 ...] so value/Z matmuls batch heads.
            pT_all = cp.tile([128, 2 * T], VD)
            for c4 in (1, 0):  # mk's chunks (4-7) land first
                sc = pp.tile([128, 1024], F32, tag="sc", bufs=2)
                for j in range(4):
                    c = c4 * 4 + j
                    nc.tensor.matmul(out=sc[:, j * 256:(j + 1) * 256],
                                     lhsT=kchunk(c),
                                     rhs=ms, start=True, stop=True)
                nc.scalar.activation(
                    out=pT_all[:, c4 * 1024:(c4 + 1) * 1024],
                    in_=sc[:], func=AF.Exp, scale=inv_sqrt_kq)

            # ---- value matmul: vo[v, (head, q)] accumulated over the 8 key
            # chunks; both heads per matmul. Contiguous runs of same-weight
            # kind keep the LDWEIGHTS pull-ahead pipelining (~108ns/MM);
            # both Z groups run before value-B so the Z->reciprocal tail
            # overlaps value-B on the PE.
            vo = pp.tile([128, 2 * NREF], F32, tag="w2", bufs=1)
            zrow = pp.tile([1, 2 * NREF], F32, tag="s1", bufs=1)
            grp_a, grp_b = [4, 5, 6, 7], [0, 1, 2, 3]
            for i, c in enumerate(grp_a):
                nc.tensor.matmul(out=vo[:],
                                 lhsT=xw[:, c * 128:(c + 1) * 128],
                                 rhs=pT_all[:, c * 256:(c + 1) * 256],
                                 start=(i == 0), stop=False)
            for i, c in enumerate(grp_a + grp_b):
                nc.tensor.matmul(out=zrow[:],
                                 lhsT=ones_col[:],
                                 rhs=pT_all[:, c * 256:(c + 1) * 256],
                                 start=(i == 0), stop=(i == TCH - 1))
            for i, c in enumerate(grp_b):
                nc.tensor.matmul(out=vo[:],
                                 lhsT=xw[:, c * 128:(c + 1) * 128],
                                 rhs=pT_all[:, c * 256:(c + 1) * 256],
                                 start=False, stop=(i == len(grp_b) - 1))

            # ---- softmax denominators: fp16 row -> column transpose via two
            # tiny fp16 matmuls -> 1/Z columns on DVE. The row copy and
            # reciprocals overlap value-B. PSUM dep-tracking is effectively
            # bank-granular and serializes cross-engine accesses in emission
            # order, so zc shares fin1's bank (whose accesses are naturally
            # chronological) and stays away from the vo bank.
            zr_sb = cp.tile([1, 2 * NREF], FP16)
            nc.vector.tensor_copy(out=zr_sb[:], in_=zrow[:])
            f1p = pp.tile([NREF, LD + 2], F32, tag="f1", bufs=1)
            fin1 = f1p[:, 0:LD]
            zc_ps = f1p[:, LD:LD + 2]
            for h in (1, 0):
                nc.tensor.matmul(out=zc_ps[:, h:h + 1],
                                 lhsT=zr_sb[:, h * 128:(h + 1) * 128],
                                 rhs=one11[:], start=True, stop=True)
            ri = cp.tile([NREF, 2], F32)
            nc.vector.reciprocal(out=ri[:], in_=zc_ps[:])

            # single DVE copy of vo (one engine touches the vo bank after
            # the matmuls — no cross-engine serialization), then parallel
            # per-head project/scale/store chains on separate PSUM banks.
            ot = cp.tile([128, 2 * NREF], VD)
            nc.vector.tensor_copy(out=ot[:], in_=vo[:])
            fin0 = pp.tile([NREF, LD], F32, tag="f0", bufs=1)
            nc.tensor.matmul(out=fin1[:], lhsT=ot[:, 128:256],
                             rhs=wo[:, LD:2 * LD], start=True, stop=True)
            nc.tensor.matmul(out=fin0[:], lhsT=ot[:, 0:128],
                             rhs=wo[:, 0:LD], start=True, stop=True)
            res1 = cp.tile([NREF, LD], FP16)
            nc.scalar.activation(out=res1[:], in_=fin1[:],
                                 func=AF.Copy, scale=ri[:, 1:2])
            nc.scalar.dma_start(out=out_d[:, 128:256], in_=res1[:])
            res0 = cp.tile([NREF, LD], FP16)
            nc.vector.tensor_scalar_mul(out=res0[:], in0=fin0[:],
                                        scalar1=ri[:, 0:1])
            nc.sync.dma_start(out=out_d[:, 0:128], in_=res0[:])

    nc.compile()
    return nc


def _get_program(vd_name=None):
    key = "f16"
    if key not in _CACHE:
        _CACHE[key] = _build_program(key)
    return _CACHE[key]


def _host_prep(ts, ys0, ys1, emb0, emb1):
    """Full k_in^T (permuted) per batch."""
    div = np.exp(np.arange(0, DT, 2, dtype=np.float32)
                 * (-np.log(10.0) / DT)).astype(np.float32)  # (32,)
    ang = 48.0 * ts[:, :, None].astype(np.float32) * div[None, None, :]
    kT = np.empty((N, KQ, T), np.float32)
    kT[:, 0:32] = np.sin(ang).transpose(0, 2, 1)
    kT[:, 32:64] = np.cos(ang).transpose(0, 2, 1)
    kT[:, 64:96] = emb0[ys0].transpose(0, 2, 1)
    kT[:, 96:128] = emb1[ys1].transpose(0, 2, 1)

    ref = np.linspace(0.0, 1.0, NREF, dtype=np.float32)
    ang_r = 48.0 * ref[:, None] * div[None, :]  # (NREF, 32)
    qT = np.empty((KQ, NREF), np.float32)
    qT[0:32] = np.sin(ang_r).T
    qT[32:64] = np.cos(ang_r).T
    qT[64:96] = emb0[100][:, None]
    qT[96:128] = emb1[50][:, None]
    return kT, qT


def _make_in_maps(ts, ys0, ys1, x, emb0, emb1, Wq, bq, Wk, bk, Wo,
                  vd_name=None):
    bf = np.float16
    ts = np.asarray(ts, np.float32)
    x = np.asarray(x, np.float32)
    emb0 = np.asarray(emb0, np.float32)
    emb1 = np.asarray(emb1, np.float32)
    ys0 = np.asarray(ys0).astype(np.int64)
    ys1 = np.asarray(ys1).astype(np.int64)

    kT, qT = _host_prep(ts, ys0, ys1, emb0, emb1)
    # KQ permutation: (sin block | cos block | emb0 | emb1) -> reference order
    perm = np.concatenate([2 * np.arange(32), 2 * np.arange(32) + 1,
                           64 + np.arange(32), 96 + np.arange(32)])
    Wq_p = np.asarray(Wq, np.float32)[perm]
    Wk_p = np.asarray(Wk, np.float32)[perm]
    bq2 = np.asarray(bq, np.float32).reshape(H, KQ)
    Wo = np.asarray(Wo, np.float32)
    # x rearranged: chunk c on cols [c*128,(c+1)*128), key t=c*128+p on part p
    xr = np.ascontiguousarray(
        x.reshape(N, TCH, 128, LD).transpose(0, 2, 1, 3).reshape(N, 128, T))

    # ms per head-pair group: m_h = (Wq_h@Wk_h^T)^T @ qT + Wk_h@bq_h —
    # weights x fixed reference queries only (no runtime data).
    ms_by_group = []
    for hg in range(2):
        cols = []
        for h in (2 * hg, 2 * hg + 1):
            Wqh = Wq_p[:, h * KQ:(h + 1) * KQ]
            Wkh = Wk_p[:, h * KQ:(h + 1) * KQ]
            WW = Wqh @ Wkh.T                       # [KQ_q, KQ_k]
            m = WW.T @ qT + (Wkh @ bq2[h])[:, None]  # [KQ_k, NREF]
            cols.append(m)
        ms_by_group.append(
            np.ascontiguousarray(np.concatenate(cols, axis=1)).astype(bf))

    in_maps = []
    for c in range(NCORES):
        b, hg = c // 2, c % 2
        # wo laid out (LD, 2*LD): local head h rows at cols [h*LD,(h+1)*LD)
        wo2 = np.ascontiguousarray(
            Wo[hg * 256:(hg + 1) * 256, :].reshape(2, LD, LD)
            .transpose(1, 0, 2).reshape(LD, 2 * LD))
        ktb = kT[b].astype(bf)
        mk = np.concatenate([ms_by_group[hg], ktb[:, T // 2:]], axis=1)
        in_maps.append(dict(
            mk=np.ascontiguousarray(mk),
            kt0=np.ascontiguousarray(ktb[:, :T // 2]),
            xw=np.ascontiguousarray(xr[b]).astype(bf),
            wo=np.ascontiguousarray(wo2).astype(bf),
        ))
    return in_maps


def kernel(ts, ys0, ys1, x, emb0, emb1, Wq, bq, Wk, bk, Wo, bo):
    in_maps = _make_in_maps(ts, ys0, ys1, x, emb0, emb1, Wq, bq, Wk, bk, Wo)
    nc = _get_program()
    res = run_bass_kernel_spmd(nc, in_maps, list(range(NCORES)))
    bo = np.asarray(bo, np.float32)
    out = np.empty((N, NREF, LD), np.float32)
    for b in range(N):
        r0 = np.asarray(res.results[2 * b]["out"], np.float32)
        r1 = np.asarray(res.results[2 * b + 1]["out"], np.float32)
        out[b] = (r0[:, :LD] + r0[:, LD:] + r1[:, :LD] + r1[:, LD:]
                  + bo[None, :])
    return out


# revision 32
# speedup vs baseline: 1.1437x; 1.1437x over previous
"""Trainium2 Bass kernel for nn_CatConLayers (multi-head cross-attention over
time/category embeddings).

Sharding: 8 cores = 4 batches x 2 head-pairs. Each core computes, for its
batch b and heads {2g, 2g+1}:
  s_c^T = k_in^T-chunk-c @ [m_0|m_1]    (kT chunk stationary, heads batched;
                                         m_h = (Wq_h@Wk_h^T)^T q_in^T + Wk_h@bq_h
                                         is pure weight preprocessing, host-fused)
  p~    = exp(s/sqrt(KQ))               (scores are tiny: no max-subtraction;
                                         the bk term cancels in the softmax)
  vo    = sum_c x_c^T @ p~_c            (value matmul, PSUM accumulation)
  Z     = ones^T @ p~ partial rows -> fp16 row, transposed to columns by two
          tiny matmuls -> 1/Z columns
  out_h = (vo_h / Z_h) @ Wo_h           (normalization folded after Wo)
Host: builds k_in^T featurization (sinusoidal time embedding + category-
embedding rows; the ACT Sin table cannot be co-resident with the Exp table,
and on-device indirect-DMA gathers measured 1.1us each), computes ms (weights
x fixed reference queries only), shards inputs, sums the two head-pair
partials per batch, adds bo.

Schedule notes (from NTFF traces): the kernel is input-DMA-latency bound at
the start, so kT is split across both HWDGE queues (sync+scalar) and issued
first; ms rides behind one half; x rides SWDGE (naturally later, needed
later). Outputs are two independent fp16 tiles on the two HWDGE queues so
neither waits on the other. All matmul operands fp16, fp32 PSUM accumulate.

The KQ dimension is permuted (sin block | cos block | emb0 | emb1) so the
interleaved sin/cos layout of the reference never has to be materialized
on-chip; Wq/Wk rows are permuted identically on host before the ms fusion.
"""

import numpy as np

import concourse.bass as bass
import concourse.mybir as mybir
import concourse.tile as tile
from concourse import bacc
from concourse.bass_utils import run_bass_kernel_spmd

# Problem shapes (hardcoded per harness contract)
N, T, H, KQ, LD, NREF, DT = 4, 1024, 4, 128, 128, 128, 64
NCORES = 8
TCH = T // 128  # 8 key chunks of 128

F32 = mybir.dt.float32
FP16 = mybir.dt.float16
AF = mybir.ActivationFunctionType

VALUE_DTYPE = "f16"  # kept for test.py compatibility

# PE warmup matmuls (N=512 each, ~427ns cold): keep the PE busy through the
# input-DMA window so the HAM clock-gate is released before the real matmuls.
WARM_CNT = 5

_CACHE = {}


def _build_program(vd_name="f16"):
    SD = VD = FP16
    nc = bacc.Bacc("TRN2", target_bir_lowering=False, debug=False,
                   num_devices=NCORES)

    # inputs: kT split across the two HWDGE queues; ms tiny behind one half;
    # x||wo on SWDGE (needed only after the first exp).
    # mk = [ms | kt chunks 4-7] — the whole critical first score group as a
    # single DMA on one queue; kt0 (chunks 0-3) arrives second via SWDGE.
    mk_d = nc.dram_tensor("mk", [KQ, 2 * NREF + T // 2], SD,
                          kind="ExternalInput")
    kt0_d = nc.dram_tensor("kt0", [KQ, T // 2], SD, kind="ExternalInput")
    xw_d = nc.dram_tensor("xw", [128, T], VD, kind="ExternalInput")
    wo_d = nc.dram_tensor("wo", [LD, 2 * LD], VD, kind="ExternalInput")
    out_d = nc.dram_tensor("out", [NREF, 2 * LD], FP16, kind="ExternalOutput")

    inv_sqrt_kq = float(1.0 / np.sqrt(KQ))

    with tile.TileContext(nc) as tc:
        # Everything is single-use: one const SBUF pool (stable addresses, no
        # ring-overlap deps) + PSUM. PSUM banks (8 x 2KB): sc 2x2 (shared
        # with the warmup tile), zrow 1, vo|zc 1, fin0 1, fin1 1.
        with tc.tile_pool(name="const", bufs=1) as cp, \
             tc.tile_pool(name="ps", bufs=2, space="PSUM") as pp:

            # ---- input DMAs. All in-flight DMA queues share the SDMA
            # engines round-robin at packet granularity, so anything issued
            # early steals bandwidth from the critical path. Only ms (sync)
            # + kt1 (scalar) go immediately; kt0/x/wo descriptor generation
            # on SWDGE is held back behind a ~1.3us dummy memset so their
            # transfers start about when kt1 finishes.
            mk = cp.tile([KQ, 2 * NREF + T // 2], SD)
            nc.scalar.dma_start(out=mk[:], in_=mk_d[:])
            ms = mk[:, 0:2 * NREF]
            delay = cp.tile([128, 1536], F32)
            nc.gpsimd.memset(delay[:], 0.0)
            kTb = cp.tile([KQ, T // 2], SD)
            nc.gpsimd.dma_start(out=kTb[:], in_=kt0_d[:])
            xw = cp.tile([128, T], VD)
            nc.gpsimd.dma_start(out=xw[:], in_=xw_d[:])
            wo = cp.tile([LD, 2 * LD], VD)
            nc.gpsimd.dma_start(out=wo[:], in_=wo_d[:])

            def kchunk(c):  # kT columns for key chunk c
                if c >= 4:
                    return mk[:, 2 * NREF + (c - 4) * 128:
                              2 * NREF + (c - 3) * 128]
                return kTb[:, c * 128:(c + 1) * 128]

            warm = cp.tile([128, 512], SD)
            nc.vector.memset(warm[:], 0.0)
            wps = pp.tile([128, 1024], F32, tag="sc", bufs=2)
            for i in range(WARM_CNT):
                nc.tensor.matmul(out=wps[:, 0:512], lhsT=warm[:, 0:128],
                                 rhs=warm[:], start=True, stop=True)

            ones_col = cp.tile([128, 1], VD)
            nc.vector.memset(ones_col[:], 1.0)
            one11 = cp.tile([1, 1], FP16)
            nc.vector.memset(one11[:], 1.0)

            # ---- scores^T + exp. p~^T layout: chunk c, head h at
            # pT_all[:, c*256 + h*128 ...] so value/Z matmuls batch heads.
            pT_all = cp.tile([128, 2 * T], VD)
            for c4 in (1, 0):  # mk's chunks (4-7) land first
                sc = pp.tile([128, 1024], F32, tag="sc", bufs=2)
                for j in range(4):
                    c = c4 * 4 + j
                    nc.tensor.matmul(out=sc[:, j * 256:(j + 1) * 256],
                                     lhsT=kchunk(c),
                                     rhs=ms, start=True, stop=True)
                nc.scalar.activation(
                    out=pT_all[:, c4 * 1024:(c4 + 1) * 1024],
                    in_=sc[:], func=AF.Exp, scale=inv_sqrt_kq)

            # ---- value matmul: vo[v, (head, q)] accumulated over the 8 key
            # chunks; both heads per matmul. Contiguous runs of same-weight
            # kind keep the LDWEIGHTS pull-ahead pipelining (~108ns/MM);
            # both Z groups run before value-B so the Z->reciprocal tail
            # overlaps value-B on the PE.
            vo = pp.tile([128, 2 * NREF], F32, tag="w2", bufs=1)
            zrow = pp.tile([1, 2 * NREF], F32, tag="s1", bufs=1)
            grp_a, grp_b = [4, 5, 6, 7], [0, 1, 2, 3]
            for i, c in enumerate(grp_a):
                nc.tensor.matmul(out=vo[:],
                                 lhsT=xw[:, c * 128:(c + 1) * 128],
                                 rhs=pT_all[:, c * 256:(c + 1) * 256],
                                 start=(i == 0), stop=False)
            for i, c in enumerate(grp_a + grp_b):
                nc.tensor.matmul(out=zrow[:],
                                 lhsT=ones_col[:],
                                 rhs=pT_all[:, c * 256:(c + 1) * 256],
                                 start=(i == 0), stop=(i == TCH - 1))
            for i, c in enumerate(grp_b):
                nc.tensor.matmul(out=vo[:],
                                 lhsT=xw[:, c * 128:(c + 1) * 128],
                                 rhs=pT_all[:, c * 256:(c + 1) * 256],
                                 start=False, stop=(i == len(grp_b) - 1))

            # ---- softmax denominators: fp16 row -> column transpose via two
            # tiny fp16 matmuls -> 1/Z columns on DVE. The row copy and
            # reciprocals overlap value-B. PSUM dep-tracking is effectively
            # bank-granular and serializes cross-engine accesses in emission
            # order, so zc shares fin1's bank (whose accesses are naturally
            # chronological) and stays away from the vo bank.
            zr_sb = cp.tile([1, 2 * NREF], FP16)
            nc.vector.tensor_copy(out=zr_sb[:], in_=zrow[:])
            f1p = pp.tile([NREF, LD + 2], F32, tag="f1", bufs=1)
            fin1 = f1p[:, 0:LD]
            zc_ps = f1p[:, LD:LD + 2]
            for h in (1, 0):
                nc.tensor.matmul(out=zc_ps[:, h:h + 1],
                                 lhsT=zr_sb[:, h * 128:(h + 1) * 128],
                                 rhs=one11[:], start=True, stop=True)
            ri = cp.tile([NREF, 2], F32)
            nc.vector.reciprocal(out=ri[:], in_=zc_ps[:])

            # single DVE copy of vo (one engine touches the vo bank after
            # the matmuls — no cross-engine serialization), then parallel
            # per-head project/scale/store chains on separate PSUM banks.
            ot = cp.tile([128, 2 * NREF], VD)
            nc.vector.tensor_copy(out=ot[:], in_=vo[:])
            fin0 = pp.tile([NREF, LD], F32, tag="f0", bufs=1)
            nc.tensor.matmul(out=fin1[:], lhsT=ot[:, 128:256],
                             rhs=wo[:, LD:2 * LD], start=True, stop=True)
            nc.tensor.matmul(out=fin0[:], lhsT=ot[:, 0:128],
                             rhs=wo[:, 0:LD], start=True, stop=True)
            res1 = cp.tile([NREF, LD], FP16)
            nc.scalar.activation(out=res1[:], in_=fin1[:],
                                 func=AF.Copy, scale=ri[:, 1:2])
            nc.scalar.dma_start(out=out_d[:, 128:256], in_=res1[:])
            res0 = cp.tile([NREF, LD], FP16)
            nc.vector.tensor_scalar_mul(out=res0[:], in0=fin0[:],
                                        scalar1=ri[:, 0:1])
            nc.sync.dma_start(out=out_d[:, 0:128], in_=res0[:])

    nc.compile()
    return nc


def _get_program(vd_name=None):
    key = "f16"
    if key not in _CACHE:
        _CACHE[key] = _build_program(key)
    return _CACHE[key]


def _host_prep(ts, ys0, ys1, emb0, emb1):
    """Full k_in^T (permuted) per batch."""
    div = np.exp(np.arange(0, DT, 2, dtype=np.float32)
                 * (-np.log(10.0) / DT)).astype(np.float32)  # (32,)
    ang = 48.0 * ts[:, :, None].astype(np.float32) * div[None, None, :]
    kT = np.empty((N, KQ, T), np.float32)
    kT[:, 0:32] = np.sin(ang).transpose(0, 2, 1)
    kT[:, 32:64] = np.cos(ang).transpose(0, 2, 1)
    kT[:, 64:96] = emb0[ys0].transpose(0, 2, 1)
    kT[:, 96:128] = emb1[ys1].transpose(0, 2, 1)

    ref = np.linspace(0.0, 1.0, NREF, dtype=np.float32)
    ang_r = 48.0 * ref[:, None] * div[None, :]  # (NREF, 32)
    qT = np.empty((KQ, NREF), np.float32)
    qT[0:32] = np.sin(ang_r).T
    qT[32:64] = np.cos(ang_r).T
    qT[64:96] = emb0[100][:, None]
    qT[96:128] = emb1[50][:, None]
    return kT, qT


def _make_in_maps(ts, ys0, ys1, x, emb0, emb1, Wq, bq, Wk, bk, Wo,
                  vd_name=None):
    bf = np.float16
    ts = np.asarray(ts, np.float32)
    x = np.asarray(x, np.float32)
    emb0 = np.asarray(emb0, np.float32)
    emb1 = np.asarray(emb1, np.float32)
    ys0 = np.asarray(ys0).astype(np.int64)
    ys1 = np.asarray(ys1).astype(np.int64)

    kT, qT = _host_prep(ts, ys0, ys1, emb0, emb1)
    # KQ permutation: (sin block | cos block | emb0 | emb1) -> reference order
    perm = np.concatenate([2 * np.arange(32), 2 * np.arange(32) + 1,
                           64 + np.arange(32), 96 + np.arange(32)])
    Wq_p = np.asarray(Wq, np.float32)[perm]
    Wk_p = np.asarray(Wk, np.float32)[perm]
    bq2 = np.asarray(bq, np.float32).reshape(H, KQ)
    Wo = np.asarray(Wo, np.float32)
    # x rearranged: chunk c on cols [c*128,(c+1)*128), key t=c*128+p on part p
    xr = np.ascontiguousarray(
        x.reshape(N, TCH, 128, LD).transpose(0, 2, 1, 3).reshape(N, 128, T))

    # ms per head-pair group: m_h = (Wq_h@Wk_h^T)^T @ qT + Wk_h@bq_h —
    # weights x fixed reference queries only (no runtime data).
    ms_by_group = []
    for hg in range(2):
        cols = []
        for h in (2 * hg, 2 * hg + 1):
            Wqh = Wq_p[:, h * KQ:(h + 1) * KQ]
            Wkh = Wk_p[:, h * KQ:(h + 1) * KQ]
            WW = Wqh @ Wkh.T                       # [KQ_q, KQ_k]
            m = WW.T @ qT + (Wkh @ bq2[h])[:, None]  # [KQ_k, NREF]
            cols.append(m)
        ms_by_group.append(
            np.ascontiguousarray(np.concatenate(cols, axis=1)).astype(bf))

    in_maps = []
    for c in range(NCORES):
        b, hg = c // 2, c % 2
        # wo laid out (LD, 2*LD): local head h rows at cols [h*LD,(h+1)*LD)
        wo2 = np.ascontiguousarray(
            Wo[hg * 256:(hg + 1) * 256, :].reshape(2, LD, LD)
            .transpose(1, 0, 2).reshape(LD, 2 * LD))
        ktb = kT[b].astype(bf)
        mk = np.concatenate([ms_by_group[hg], ktb[:, T // 2:]], axis=1)
        in_maps.append(dict(
            mk=np.ascontiguousarray(mk),
            kt0=np.ascontiguousarray(ktb[:, :T // 2]),
            xw=np.ascontiguousarray(xr[b]).astype(bf),
            wo=np.ascontiguousarray(wo2).astype(bf),
        ))
    return in_maps


def kernel(ts, ys0, ys1, x, emb0, emb1, Wq, bq, Wk, bk, Wo, bo):
    in_maps = _make_in_maps(ts, ys0, ys1, x, emb0, emb1, Wq, bq, Wk, bk, Wo)
    nc = _get_program()
    res = run_bass_kernel_spmd(nc, in_maps, list(range(NCORES)))
    bo = np.asarray(bo, np.float32)
    out = np.empty((N, NREF, LD), np.float32)
    for b in range(N):
        r0 = np.asarray(res.results[2 * b]["out"], np.float32)
        r1 = np.asarray(res.results[2 * b + 1]["out"], np.float32)
        out[b] = (r0[:, :LD] + r0[:, LD:] + r1[:, :LD] + r1[:, LD:]
                  + bo[None, :])
    return out
